# revision 1
# baseline (speedup 1.0000x reference)
"""Tensor-parallel attention kernel for Trainium2 (8 NeuronCores).

Problem: B=1, L=2048, D=4096, H=32 q-heads, KV=8 kv-heads, HD=128,
partial rotary ROT=64, causal additive mask, o-projection.

Sharding: TP-8 over heads. Core c owns q-heads 4c..4c+3 and kv-head c
(column shard of w_qkv), plus the matching row shard of w_o. Each core
computes a full [L, D] partial of the output; the host sums the 8
partials (the cross-core reduction of the row-sharded o-projection).

Everything on-chip runs in "transposed" orientation so every matmul
contracts over the partition dim with zero on-chip activation
transposes:
  qkvT[col, L] = w_qkv.T @ x.T          (w stationary, xT streamed)
  rope:  qT' = qT * cosE + (P @ qT) * sinE   (P = rotate-half matrix on PE)
  ST[k, q]   = kT_tile.T @ qT            (one matmul per k-tile, K=HD=128)
  PT         = exp(ST + maskT)           (no max subtraction; exp(-1e9)=0)
  den[*, q]  = ones.T @ PT               (ones-matmul, accumulated over k)
  oT[d, q]   = V_tile.T @ PT             (V from a one-time PE transpose of vT)
  out[l, e]  = (oT/den).T @ w_o_shard    (partial; summed across cores on host)
"""

import sys

for _p in ("/opt/trn_rl_repo", "/root/.axon_site/_ro/trn_rl_repo"):
    if _p not in sys.path:
        sys.path.append(_p)

import numpy as np

B, L, D = 1, 2048, 4096
H, KV, HD = 32, 8, 128
ROT = 64
SCALE = HD ** -0.5
NEG = -1e9
NCORES = 8
HPC = H // NCORES          # q-heads per core (4)
CPC = HPC * HD + 2 * HD    # w_qkv columns per core (768)
NDT = D // 128             # contraction tiles over D (32)
NKT = L // 128             # k tiles (16)
NJQ = L // 512             # 512-wide q blocks (4)
XBLK = 256                 # L-block width in the qkv phase

_cache = {}


def _build(causal: bool):
    import concourse.mybir as mybir
    import concourse.tile as tile
    from concourse import bacc

    F32 = mybir.dt.float32
    F32R = mybir.dt.float32r
    EXP = mybir.ActivationFunctionType.Exp

    nc = bacc.Bacc("TRN2", target_bir_lowering=False, debug=False)

    xt = nc.dram_tensor("xt", [D, L], F32, kind="ExternalInput").ap()
    wqkv = nc.dram_tensor("wqkv", [D, CPC], F32, kind="ExternalInput").ap()
    wo = nc.dram_tensor("wo", [HPC * HD, D], F32, kind="ExternalInput").ap()
    cos_e = nc.dram_tensor("cos_e", [2, 128, L], F32, kind="ExternalInput").ap()
    sin_e = nc.dram_tensor("sin_e", [2, 128, L], F32, kind="ExternalInput").ap()
    consts = nc.dram_tensor("consts", [128, 384], F32, kind="ExternalInput").ap()
    if causal:
        # block-diagonal strip of maskT: [jq, ktile-in-block, 128, 512]
        mask_d = nc.dram_tensor("mask_d", [NJQ, 4, 128, 512], F32,
                                kind="ExternalInput").ap()
    else:
        mask_t = nc.dram_tensor("mask_t", [L, L], F32, kind="ExternalInput").ap()
    out_p = nc.dram_tensor("out_p", [L, D], F32, kind="ExternalOutput").ap()

    qt_r = nc.dram_tensor("qt_r", [HPC, 128, L], F32R)  # roped qT, internal

    xt_r = xt.rearrange("(dt p) l -> p dt l", p=128).bitcast(F32R)
    wqkv_r = wqkv.rearrange("(dt p) c -> p dt c", p=128).bitcast(F32R)
    wo_r = wo.rearrange("(h p) e -> p h e", p=128).bitcast(F32R)

    with tile.TileContext(nc) as tc:
        with tc.tile_pool(name="persist", bufs=1) as persist:
            kt_sb = persist.tile([128, L], F32R, tag="kt")
            v_sb = persist.tile([128, NKT, 128], F32R, tag="v")
            cst = persist.tile([128, 384], F32R, tag="cst")
            nc.sync.dma_start(out=cst, in_=consts.bitcast(F32R))
            ident = cst[:, 0:128]
            ones = cst[:, 128:256]
            pmat_t = cst[:, 256:384]

            # ---------------- Phase 1: qkv projection + rope ----------------
            with tc.tile_pool(name="wq", bufs=1) as wqp, \
                 tc.tile_pool(name="xb", bufs=2) as xbp, \
                 tc.tile_pool(name="tabs", bufs=1) as tabs, \
                 tc.tile_pool(name="stage", bufs=3) as stage, \
                 tc.tile_pool(name="vtmp", bufs=1) as vtmp, \
                 tc.tile_pool(name="ps1", bufs=4, space="PSUM") as ps1, \
                 tc.tile_pool(name="psr", bufs=2, space="PSUM") as psr:
                wq_sb = wqp.tile([128, NDT, CPC], F32R)
                vt_sb = vtmp.tile([128, L], F32R)

                for lb in range(L // XBLK):
                    ls = slice(lb * XBLK, (lb + 1) * XBLK)
                    xblk = xbp.tile([128, NDT, XBLK], F32R, tag="xblk")
                    if lb == 0:
                        # interleave first x chunks with weight slabs so the
                        # first accumulation can start as data arrives
                        for sl in range(8):
                            ss = slice(sl * NDT // 8, (sl + 1) * NDT // 8)
                            nc.gpsimd.dma_start(out=xblk[:, ss, :], in_=xt_r[:, ss, ls])
                            weng = nc.sync if sl % 2 == 0 else nc.scalar
                            weng.dma_start(out=wq_sb[:, ss, :], in_=wqkv_r[:, ss, :])
                    else:
                        nc.gpsimd.dma_start(out=xblk, in_=xt_r[:, :, ls])
                    cosb = tabs.tile([128, 2, XBLK], F32, tag="cosb")
                    sinb = tabs.tile([128, 2, XBLK], F32, tag="sinb")
                    nc.sync.dma_start(out=cosb, in_=cos_e[:, :, ls].rearrange("t p l -> p t l"))
                    nc.sync.dma_start(out=sinb, in_=sin_e[:, :, ls].rearrange("t p l -> p t l"))
                    for ct in range(6):
                        acc = ps1.tile([128, XBLK], F32, tag="acc")
                        for dti in range(NDT):
                            nc.tensor.matmul(
                                out=acc,
                                lhsT=wq_sb[:, dti, ct * 128:(ct + 1) * 128],
                                rhs=xblk[:, dti, :],
                                start=(dti == 0), stop=(dti == NDT - 1))
                        if ct == 5:
                            # v: copy to vT staging, then transpose this
                            # block's two k-tiles into resident V
                            nc.scalar.copy(out=vt_sb[:, ls], in_=acc)
                            for kk in range(2):
                                i = 2 * lb + kk
                                tp = psr.tile([128, 128], F32R, tag="vtp")
                                nc.tensor.transpose(
                                    tp, vt_sb[:, i * 128:(i + 1) * 128], ident)
                                nc.vector.tensor_copy(v_sb[:, i, :], tp)
                            continue
                        # rope for q (ct 0..3, scaled tables) and k (ct 4)
                        ti = 0 if ct < 4 else 1
                        s_sb = stage.tile([128, XBLK], F32R, tag="s_sb")
                        nc.scalar.copy(out=s_sb, in_=acc)
                        rot = psr.tile([128, XBLK], F32, tag="rot")
                        nc.tensor.matmul(out=rot, lhsT=pmat_t, rhs=s_sb,
                                         start=True, stop=True)
                        dst = kt_sb[:, ls] if ct == 4 else None
                        if dst is None:
                            dtile = stage.tile([128, XBLK], F32R, tag="dtile")
                        else:
                            dtile = dst
                        nc.vector.tensor_mul(dtile, s_sb, cosb[:, ti, :])
                        m2 = stage.tile([128, XBLK], F32R, tag="m2")
                        nc.vector.tensor_mul(m2, rot, sinb[:, ti, :])
                        nc.vector.tensor_add(dtile, dtile, m2)
                        if dst is None:
                            nc.sync.dma_start(out=qt_r[ct][:, ls], in_=dtile)


            # ---------------- Phases 2+3 ----------------
            late_cm = tc.tile_pool(name="late", bufs=1)
            late = late_cm.__enter__()
            otn_sb = late.tile([128, HPC, L], F32R, tag="otn")

            # ---------------- Phase 2: attention ----------------
            with tc.tile_pool(name="qb", bufs=3) as qbp, \
                 tc.tile_pool(name="mb", bufs=2) as mbp, \
                 tc.tile_pool(name="pt", bufs=6) as ptp, \
                 tc.tile_pool(name="rdp", bufs=2) as rdp, \
                 tc.tile_pool(name="ps_st", bufs=4, space="PSUM") as ps_st, \
                 tc.tile_pool(name="ps_acc", bufs=2, space="PSUM") as ps_acc:
                for jq in range(NJQ):
                    qs = slice(jq * 512, (jq + 1) * 512)
                    nkt = 4 * (jq + 1) if causal else NKT
                    diag0 = 4 * jq
                    if causal:
                        mblk = mbp.tile([128, 4, 512], F32, tag="mblk")
                        nc.sync.dma_start(
                            out=mblk, in_=mask_d[jq].rearrange("kt p q -> p kt q"))
                    else:
                        mblk = mbp.tile([128, NKT, 512], F32, tag="mblk")
                        nc.sync.dma_start(
                            out=mblk,
                            in_=mask_t[:, qs].rearrange("(kt p) q -> p kt q", p=128))
                    for h in range(HPC):
                        qblk = qbp.tile([128, 512], F32R, tag="qblk")
                        nc.sync.dma_start(out=qblk, in_=qt_r[h][:, qs])
                        den = ps_acc.tile([128, 512], F32, tag="den")
                        ot = ps_acc.tile([128, 512], F32, tag="ot")
                        for i in range(nkt):
                            st = ps_st.tile([128, 512], F32, tag="st")
                            nc.tensor.matmul(
                                out=st, lhsT=kt_sb[:, i * 128:(i + 1) * 128],
                                rhs=qblk, start=True, stop=True)
                            if causal:
                                if i >= diag0:
                                    nc.vector.tensor_add(st, st, mblk[:, i - diag0, :])
                            else:
                                nc.vector.tensor_add(st, st, mblk[:, i, :])
                            pt = ptp.tile([128, 512], F32R, tag="pt")
                            nc.scalar.activation(pt, st, EXP)
                            nc.tensor.matmul(out=den, lhsT=ones, rhs=pt,
                                             start=(i == 0), stop=(i == nkt - 1))
                            nc.tensor.matmul(out=ot, lhsT=v_sb[:, i, :], rhs=pt,
                                             start=(i == 0), stop=(i == nkt - 1))
                        rd = rdp.tile([128, 512], F32, tag="rd")
                        nc.vector.reciprocal_approx_fast(out=rd, in_=den)
                        nc.vector.tensor_mul(otn_sb[:, h, qs], ot, rd)


            # ---------------- Phase 3: o-projection ----------------
            with tc.tile_pool(name="wob", bufs=2) as wop, \
                 tc.tile_pool(name="ost", bufs=6) as ostp, \
                 tc.tile_pool(name="ps3", bufs=6, space="PSUM") as ps3:
                for et in range(D // 512):
                    es = slice(et * 512, (et + 1) * 512)
                    wob = wop.tile([128, HPC, 512], F32R, tag="wob")
                    nc.gpsimd.dma_start(out=wob, in_=wo_r[:, :, es])
                    for lt in range(L // 128):
                        acc = ps3.tile([128, 512], F32, tag="acc3")
                        for h in range(HPC):
                            nc.tensor.matmul(
                                out=acc,
                                lhsT=otn_sb[:, h, lt * 128:(lt + 1) * 128],
                                rhs=wob[:, h, :],
                                start=(h == 0), stop=(h == HPC - 1))
                        ost = ostp.tile([128, 512], F32, tag="ost")
                        if lt % 2 == 0:
                            nc.vector.tensor_copy(ost, acc)
                        else:
                            nc.scalar.copy(out=ost, in_=acc)
                        nc.sync.dma_start(out=out_p[lt * 128:(lt + 1) * 128, es], in_=ost)

            late_cm.__exit__(None, None, None)

    nc.compile()
    return nc


def _host_inputs(x, attention_mask, cos, sin, w_qkv, w_o, causal):
    """Build the 8 per-core input maps (all fp32, C-contiguous)."""
    xt = np.ascontiguousarray(x[0].T)                     # [D, L]
    q_pos = H * HD
    kv_pos = q_pos + KV * HD

    # extended rope tables [2, 128, L]: slot 0 = q (scale folded), slot 1 = k
    # row d<64: cos[l, d]; row d>=64: 1.0 (cos) / 0.0 (sin)
    cos_t = cos.T.astype(np.float32)                      # [ROT, L]
    sin_t = sin.T.astype(np.float32)
    cos_e = np.empty((2, 128, L), np.float32)
    sin_e = np.zeros((2, 128, L), np.float32)
    cos_e[0, :ROT] = cos_t * SCALE
    cos_e[0, ROT:] = SCALE
    cos_e[1, :ROT] = cos_t
    cos_e[1, ROT:] = 1.0
    sin_e[0, :ROT] = sin_t * SCALE
    sin_e[1, :ROT] = sin_t

    # consts [128, 384] = [identity | ones | pmat_t]
    # pmat_t[d, d'] = Pmat[d', d]; rot[d'] = -x[d'+32] (d'<32), x[d'-32] (32<=d'<64)
    pmat = np.zeros((128, 128), np.float32)
    for dp in range(32):
        pmat[dp, dp + 32] = -1.0
    for dp in range(32, 64):
        pmat[dp, dp - 32] = 1.0
    consts = np.concatenate(
        [np.eye(128, dtype=np.float32), np.ones((128, 128), np.float32), pmat.T], axis=1)

    mask2d = np.ascontiguousarray(attention_mask[0, 0])   # [L(q), L(k)]
    if causal:
        mask_t_full = None
        # diagonal 512x512 blocks of maskT, split into 128-row k strips
        mask_d = np.empty((NJQ, 4, 128, 512), np.float32)
        mt = mask2d.T                                     # [k, q]
        for jq in range(NJQ):
            blk = mt[jq * 512:(jq + 1) * 512, jq * 512:(jq + 1) * 512]
            mask_d[jq] = blk.reshape(4, 128, 512)
        mask_d = np.ascontiguousarray(mask_d)
    else:
        mask_t_full = np.ascontiguousarray(mask2d.T)      # [k, q]
        mask_d = None

    in_maps = []
    for c in range(NCORES):
        cols = []
        for j in range(HPC):
            h = c * HPC + j
            cols.append(w_qkv[:, h * HD:(h + 1) * HD])
        cols.append(w_qkv[:, q_pos + c * HD:q_pos + (c + 1) * HD])
        cols.append(w_qkv[:, kv_pos + c * HD:kv_pos + (c + 1) * HD])
        wqkv_c = np.ascontiguousarray(np.concatenate(cols, axis=1))  # [D, 768]
        wo_c = np.ascontiguousarray(
            w_o[c * HPC * HD:(c + 1) * HPC * HD, :])                 # [512, D]
        m = {"xt": xt, "wqkv": wqkv_c, "wo": wo_c,
             "cos_e": cos_e, "sin_e": sin_e, "consts": consts}
        if causal:
            m["mask_d"] = mask_d
        else:
            m["mask_t"] = mask_t_full
        in_maps.append(m)
    return in_maps


def _is_causal(mask2d):
    expected = np.where(
        np.tril(np.ones((L, L), dtype=bool)), np.float32(0.0), np.float32(NEG))
    return mask2d.shape == (L, L) and np.array_equal(mask2d, expected)


def kernel(x, attention_mask, cos, sin, w_qkv, w_o, _trace=False):
    from concourse.bass_utils import run_bass_kernel_spmd

    x = np.asarray(x, dtype=np.float32)
    attention_mask = np.asarray(attention_mask, dtype=np.float32)
    cos = np.asarray(cos, dtype=np.float32)
    sin = np.asarray(sin, dtype=np.float32)
    w_qkv = np.asarray(w_qkv, dtype=np.float32)
    w_o = np.asarray(w_o, dtype=np.float32)

    causal = _is_causal(attention_mask[0, 0])
    if causal not in _cache:
        _cache[causal] = _build(causal)
    nc = _cache[causal]

    in_maps = _host_inputs(x, attention_mask, cos, sin, w_qkv, w_o, causal)
    try:
        res = run_bass_kernel_spmd(nc, in_maps, list(range(NCORES)), trace=_trace)
    except Exception:
        # transient device errors (e.g. NRT_EXEC_UNIT_UNRECOVERABLE) usually
        # clear on retry
        res = run_bass_kernel_spmd(nc, in_maps, list(range(NCORES)), trace=_trace)
    out = np.zeros((L, D), np.float64)
    for c in range(NCORES):
        out += res.results[c]["out_p"].astype(np.float64)
    if _trace:
        kernel._last_exec_time_ns = res.exec_time_ns
    return out.astype(np.float32).reshape(B, L, D)



# revision 5
# speedup vs baseline: 1.2969x; 1.2969x over previous
"""Tensor-parallel attention kernel for Trainium2 (8 NeuronCores).

Problem: B=1, L=2048, D=4096, H=32 q-heads, KV=8 kv-heads, HD=128,
partial rotary ROT=64, causal additive mask, o-projection.

Sharding: TP-8 over heads. Core c owns q-heads 4c..4c+3 and kv-head c
(column shard of w_qkv), plus the matching row shard of w_o. Each core
computes a full [L, D] partial of the output; the host sums the 8
partials (the cross-core reduction of the row-sharded o-projection).

All matmul operands are fp16 (PSUM accumulation stays fp32): 2-byte
operands stream through the PE array ~2x faster than fp32 and halve
HBM traffic. Softmax uses a constant exp shift (exp(s-4)) so fp16
probability tiles can never overflow; the shift cancels in the
normalization. The causal mask is applied as a multiplicative 0/1
pattern on the four diagonal-tile offsets (tiny resident table)
instead of DMA-ing a [L, L] additive mask.

Everything on-chip runs in "transposed" orientation so every matmul
contracts over the partition dim with zero on-chip activation
transposes:
  qkvT[col, L] = w_qkv.T @ x.T          (w stationary, xT streamed)
  rope:  qT' = qT * cosE + (P @ qT) * sinE   (P = rotate-half matrix on PE)
  ST[k, q]   = kT_tile.T @ qT            (one matmul per k-tile, K=HD=128)
  PT         = exp(ST - 4) (* diag 0/1 mask on diagonal tiles)
  den[*, q]  = ones.T @ PT               (ones-matmul, accumulated over k)
  oT[d, q]   = V_tile.T @ PT             (V from a one-time PE transpose of vT)
  out[l, e]  = (oT/den).T @ w_o_shard    (partial; summed across cores on host)
"""

import sys

for _p in ("/opt/trn_rl_repo", "/root/.axon_site/_ro/trn_rl_repo"):
    if _p not in sys.path:
        sys.path.append(_p)

import numpy as np

B, L, D = 1, 2048, 4096
H, KV, HD = 32, 8, 128
ROT = 64
SCALE = HD ** -0.5
NEG = -1e9
NCORES = 8
HPC = H // NCORES          # q-heads per core (4)
CPC = HPC * HD + 2 * HD    # w_qkv columns per core (768)
NDT = D // 128             # contraction tiles over D (32)
NKT = L // 128             # k tiles (16)
NJQ = L // 512             # 512-wide q blocks (4)
XBLK = 512                 # L-block width in the qkv phase
NLB = L // XBLK            # 4
EXPSHIFT = -4.0            # softmax exp shift; cancels in normalization

_cache = {}


def _build(causal: bool):
    import concourse.mybir as mybir
    import concourse.tile as tile
    from concourse import bacc

    F32 = mybir.dt.float32
    F16 = mybir.dt.float16
    EXP = mybir.ActivationFunctionType.Exp

    nc = bacc.Bacc("TRN2", target_bir_lowering=False, debug=False)

    xt = nc.dram_tensor("xt", [D, L], F16, kind="ExternalInput").ap()
    wqkv = nc.dram_tensor("wqkv", [D, CPC], F16, kind="ExternalInput").ap()
    wo = nc.dram_tensor("wo", [HPC * HD, D], F16, kind="ExternalInput").ap()
    cos_e = nc.dram_tensor("cos_e", [2, 128, L], F16, kind="ExternalInput").ap()
    sin_e = nc.dram_tensor("sin_e", [2, 128, L], F16, kind="ExternalInput").ap()
    consts = nc.dram_tensor("consts", [128, 384], F16, kind="ExternalInput").ap()
    if causal:
        # 0/1 multiplier for the 4 diagonal k-tile offsets: [128, 4, 512]
        dmask = nc.dram_tensor("dmask", [128, 4, 512], F16,
                               kind="ExternalInput").ap()
    else:
        mask_t = nc.dram_tensor("mask_t", [L, L], F16, kind="ExternalInput").ap()
    out_p = nc.dram_tensor("out_p", [L, D], F16, kind="ExternalOutput").ap()

    xt_r = xt.rearrange("(dt p) l -> p dt l", p=128)
    wqkv_r = wqkv.rearrange("(dt p) c -> p dt c", p=128)
    wo_r = wo.rearrange("(h p) e -> p h e", p=128)

    with tile.TileContext(nc) as tc:
        with tc.tile_pool(name="persist", bufs=1) as persist:
            kt_sb = persist.tile([128, L], F16, tag="kt")
            v_sb = persist.tile([128, NKT, 128], F16, tag="v")
            qt_sb = persist.tile([128, HPC, L], F16, tag="qt")
            otn_sb = persist.tile([128, HPC, L], F16, tag="otn")
            cst = persist.tile([128, 384], F16, tag="cst")
            nc.sync.dma_start(out=cst, in_=consts)
            ident = cst[:, 0:128]
            ones = cst[:, 128:256]
            pmat_t = cst[:, 256:384]
            if causal:
                dmask_sb = persist.tile([128, 4, 512], F16, tag="dmask")
                nc.sync.dma_start(out=dmask_sb, in_=dmask)
            expb = persist.tile([128, 1], F32, tag="expb")
            nc.gpsimd.memset(expb, EXPSHIFT)

            # ---------------- Phase 1: qkv projection + rope ----------------
            with tc.tile_pool(name="wq", bufs=1) as wqp, \
                 tc.tile_pool(name="xb", bufs=2) as xbp, \
                 tc.tile_pool(name="tabs", bufs=2) as tabs, \
                 tc.tile_pool(name="stage", bufs=3) as stage, \
                 tc.tile_pool(name="vtmp", bufs=2) as vtmp, \
                 tc.tile_pool(name="ps1", bufs=3, space="PSUM") as ps1, \
                 tc.tile_pool(name="psr", bufs=2, space="PSUM") as psr:
                wq_sb = wqp.tile([128, NDT, CPC], F16)

                # deferred tail-work (PE rot matmul / v transposes) per (lb,
                # ct), emitted one matmul-group later so the PE never stalls
                # waiting on the ACT copy of the previous group's PSUM.
                pending = []

                def flush_pending():
                    while pending:
                        pending.pop(0)()

                for lb in range(NLB):
                    ls = slice(lb * XBLK, (lb + 1) * XBLK)
                    xblk = xbp.tile([128, NDT, XBLK], F16, tag="xblk")
                    if lb == 0:
                        # interleave first x chunks with weight slabs so the
                        # first accumulation can start as data arrives
                        for sl in range(8):
                            ss = slice(sl * NDT // 8, (sl + 1) * NDT // 8)
                            nc.gpsimd.dma_start(out=xblk[:, ss, :], in_=xt_r[:, ss, ls])
                            weng = nc.sync if sl % 2 == 0 else nc.scalar
                            weng.dma_start(out=wq_sb[:, ss, :], in_=wqkv_r[:, ss, :])
                    else:
                        nc.gpsimd.dma_start(out=xblk, in_=xt_r[:, :, ls])
                    cosb = tabs.tile([128, 2, XBLK], F16, tag="cosb")
                    sinb = tabs.tile([128, 2, XBLK], F16, tag="sinb")
                    nc.sync.dma_start(out=cosb, in_=cos_e[:, :, ls].rearrange("t p l -> p t l"))
                    nc.sync.dma_start(out=sinb, in_=sin_e[:, :, ls].rearrange("t p l -> p t l"))
                    for ct in range(6):
                        acc = ps1.tile([128, XBLK], F32, tag="acc")
                        for dti in range(NDT):
                            nc.tensor.matmul(
                                out=acc,
                                lhsT=wq_sb[:, dti, ct * 128:(ct + 1) * 128],
                                rhs=xblk[:, dti, :],
                                start=(dti == 0), stop=(dti == NDT - 1))
                        if ct == 5:
                            # v: copy to vT staging; transpose deferred
                            vt = vtmp.tile([128, XBLK], F16, tag="vt")
                            nc.scalar.copy(out=vt, in_=acc)

                            def fin_v(lb=lb, vt=vt):
                                for kk in range(XBLK // 128):
                                    i = (XBLK // 128) * lb + kk
                                    tp = psr.tile([128, 128], F16, tag="vtp")
                                    nc.tensor.transpose(
                                        tp, vt[:, kk * 128:(kk + 1) * 128], ident)
                                    nc.vector.tensor_copy(v_sb[:, i, :], tp)

                            pending.append(fin_v)
                            continue
                        # rope for q (ct 0..3, scaled tables) and k (ct 4)
                        ti = 0 if ct < 4 else 1
                        s_sb = stage.tile([128, XBLK], F16, tag="s_sb")
                        nc.scalar.copy(out=s_sb, in_=acc)

                        def fin_rope(ct=ct, s_sb=s_sb, cosb=cosb, sinb=sinb,
                                     ti=ti, ls=ls):
                            rot = psr.tile([128, XBLK], F32, tag="rot")
                            nc.tensor.matmul(out=rot, lhsT=pmat_t, rhs=s_sb,
                                             start=True, stop=True)
                            dst = kt_sb[:, ls] if ct == 4 else qt_sb[:, ct, ls]
                            nc.vector.tensor_mul(dst, s_sb, cosb[:, ti, :])
                            m2 = stage.tile([128, XBLK], F16, tag="m2")
                            nc.vector.tensor_mul(m2, rot, sinb[:, ti, :])
                            nc.vector.tensor_add(dst, dst, m2)

                        flush_pending()
                        pending.append(fin_rope)
                flush_pending()

            # wo shard stays resident through phases 2+3
            with tc.tile_pool(name="wop", bufs=1) as wop:
                wo_sb = wop.tile([128, HPC, D], F16)
                nc.gpsimd.dma_start(out=wo_sb, in_=wo_r)

                # ---------------- Phase 2: attention ----------------
                with tc.tile_pool(name="ptp", bufs=4) as ptp, \
                     tc.tile_pool(name="mb", bufs=2) as mbp, \
                     tc.tile_pool(name="rdp", bufs=2) as rdp, \
                     tc.tile_pool(name="ps_st", bufs=2, space="PSUM") as ps_st, \
                     tc.tile_pool(name="ps_acc", bufs=2, space="PSUM") as ps_acc:
                    for jq in range(NJQ):
                        qs = slice(jq * 512, (jq + 1) * 512)
                        nkt = 4 * (jq + 1) if causal else NKT
                        diag0 = 4 * jq
                        if not causal:
                            mblk = mbp.tile([128, NKT, 512], F16, tag="mblk")
                            nc.sync.dma_start(
                                out=mblk,
                                in_=mask_t[:, qs].rearrange("(kt p) q -> p kt q", p=128))
                        for h in range(HPC):
                            den = ps_acc.tile([128, 512], F32, tag="den")
                            ot = ps_acc.tile([128, 512], F32, tag="ot")
                            npair = nkt // 2
                            pts = {}

                            def emit_den_ot(g, den=den, ot=ot, nkt=nkt,
                                            diag0=diag0, pts=pts):
                                pt = pts.pop(g)
                                for t in (0, 1):
                                    i = 2 * g + t
                                    if causal and i >= diag0:
                                        nc.vector.tensor_mul(
                                            pt[:, t, :], pt[:, t, :],
                                            dmask_sb[:, i - diag0, :])
                                    nc.tensor.matmul(
                                        out=den, lhsT=ones, rhs=pt[:, t, :],
                                        start=(i == 0), stop=(i == nkt - 1))
                                    nc.tensor.matmul(
                                        out=ot, lhsT=v_sb[:, i, :], rhs=pt[:, t, :],
                                        start=(i == 0), stop=(i == nkt - 1))

                            for g in range(npair):
                                st = ps_st.tile([128, 2, 512], F32, tag="st")
                                for t in (0, 1):
                                    i = 2 * g + t
                                    nc.tensor.matmul(
                                        out=st[:, t, :],
                                        lhsT=kt_sb[:, i * 128:(i + 1) * 128],
                                        rhs=qt_sb[:, h, qs],
                                        start=True, stop=True)
                                if not causal:
                                    for t in (0, 1):
                                        nc.vector.tensor_add(
                                            st[:, t, :], st[:, t, :],
                                            mblk[:, 2 * g + t, :])
                                pt = ptp.tile([128, 2, 512], F16, tag="pt")
                                nc.scalar.activation(pt, st, EXP, bias=expb)
                                pts[g] = pt
                                if g > 0:
                                    emit_den_ot(g - 1)
                            emit_den_ot(npair - 1)
                            rd = rdp.tile([128, 512], F32, tag="rd")
                            nc.vector.reciprocal_approx_fast(out=rd, in_=den)
                            nc.vector.tensor_mul(otn_sb[:, h, qs], ot, rd)

                # ---------------- Phase 3: o-projection ----------------
                with tc.tile_pool(name="ost", bufs=2) as ostp, \
                     tc.tile_pool(name="ps3", bufs=8, space="PSUM") as ps3:
                    for lt in range(L // 128):
                        lsl = slice(lt * 128, (lt + 1) * 128)
                        ostage = ostp.tile([128, D // 512, 512], F16, tag="ostage")
                        for eg in range(2):
                            accs = [ps3.tile([128, 512], F32, tag="acc3",
                                             name=f"acc3_{lt}_{eg}_{k}")
                                    for k in range(4)]
                            for h in range(HPC):
                                for e4 in range(4):
                                    et = eg * 4 + e4
                                    nc.tensor.matmul(
                                        out=accs[e4],
                                        lhsT=otn_sb[:, h, lsl],
                                        rhs=wo_sb[:, h, et * 512:(et + 1) * 512],
                                        start=(h == 0), stop=(h == HPC - 1))
                            for e4 in range(4):
                                et = eg * 4 + e4
                                if e4 % 2 == 0:
                                    nc.vector.tensor_copy(ostage[:, et, :], accs[e4])
                                else:
                                    nc.scalar.copy(out=ostage[:, et, :], in_=accs[e4])
                        nc.sync.dma_start(out=out_p[lsl, :], in_=ostage)

    nc.compile()
    return nc


def _host_inputs(x, attention_mask, cos, sin, w_qkv, w_o, causal):
    """Build the 8 per-core input maps (fp16, C-contiguous)."""
    F16 = np.float16
    xt = np.ascontiguousarray(x[0].T).astype(F16)         # [D, L]
    q_pos = H * HD
    kv_pos = q_pos + KV * HD

    # extended rope tables [2, 128, L]: slot 0 = q (scale folded), slot 1 = k
    # row d<64: cos[l, d]; row d>=64: 1.0 (cos) / 0.0 (sin)
    cos_t = cos.T.astype(np.float32)                      # [ROT, L]
    sin_t = sin.T.astype(np.float32)
    cos_e = np.empty((2, 128, L), np.float32)
    sin_e = np.zeros((2, 128, L), np.float32)
    cos_e[0, :ROT] = cos_t * SCALE
    cos_e[0, ROT:] = SCALE
    cos_e[1, :ROT] = cos_t
    cos_e[1, ROT:] = 1.0
    sin_e[0, :ROT] = sin_t * SCALE
    sin_e[1, :ROT] = sin_t
    cos_e = cos_e.astype(F16)
    sin_e = sin_e.astype(F16)

    # consts [128, 384] = [identity | ones | pmat_t]
    pmat = np.zeros((128, 128), np.float32)
    for dp in range(32):
        pmat[dp, dp + 32] = -1.0
    for dp in range(32, 64):
        pmat[dp, dp - 32] = 1.0
    consts = np.concatenate(
        [np.eye(128, dtype=np.float32), np.ones((128, 128), np.float32),
         pmat.T], axis=1).astype(F16)

    mask2d = np.ascontiguousarray(attention_mask[0, 0])   # [L(q), L(k)]
    if causal:
        mask_t_full = None
        # 0/1 visibility for diagonal k-tiles: offset o -> visible iff
        # q_local >= k_local + 128*o  (pt layout is [k, o, q])
        kloc = np.arange(128)[:, None]
        qloc = np.arange(512)[None, :]
        dmask = np.empty((128, 4, 512), np.float32)
        for o in range(4):
            dmask[:, o, :] = (qloc >= kloc + 128 * o)
        dmask = np.ascontiguousarray(dmask.astype(F16))
    else:
        mask_t_full = np.ascontiguousarray(mask2d.T).astype(F16)  # [k, q]
        dmask = None

    in_maps = []
    for c in range(NCORES):
        cols = []
        for j in range(HPC):
            h = c * HPC + j
            cols.append(w_qkv[:, h * HD:(h + 1) * HD])
        cols.append(w_qkv[:, q_pos + c * HD:q_pos + (c + 1) * HD])
        cols.append(w_qkv[:, kv_pos + c * HD:kv_pos + (c + 1) * HD])
        wqkv_c = np.ascontiguousarray(
            np.concatenate(cols, axis=1)).astype(F16)     # [D, 768]
        wo_c = np.ascontiguousarray(
            w_o[c * HPC * HD:(c + 1) * HPC * HD, :]).astype(F16)  # [512, D]
        m = {"xt": xt, "wqkv": wqkv_c, "wo": wo_c,
             "cos_e": cos_e, "sin_e": sin_e, "consts": consts}
        if causal:
            m["dmask"] = dmask
        else:
            m["mask_t"] = mask_t_full
        in_maps.append(m)
    return in_maps


def _is_causal(mask2d):
    expected = np.where(
        np.tril(np.ones((L, L), dtype=bool)), np.float32(0.0), np.float32(NEG))
    return mask2d.shape == (L, L) and np.array_equal(mask2d, expected)


def kernel(x, attention_mask, cos, sin, w_qkv, w_o, _trace=False):
    from concourse.bass_utils import run_bass_kernel_spmd

    x = np.asarray(x, dtype=np.float32)
    attention_mask = np.asarray(attention_mask, dtype=np.float32)
    cos = np.asarray(cos, dtype=np.float32)
    sin = np.asarray(sin, dtype=np.float32)
    w_qkv = np.asarray(w_qkv, dtype=np.float32)
    w_o = np.asarray(w_o, dtype=np.float32)

    causal = _is_causal(attention_mask[0, 0])
    if causal not in _cache:
        _cache[causal] = _build(causal)
    nc = _cache[causal]

    in_maps = _host_inputs(x, attention_mask, cos, sin, w_qkv, w_o, causal)
    try:
        res = run_bass_kernel_spmd(nc, in_maps, list(range(NCORES)), trace=_trace)
    except Exception:
        # transient device errors (e.g. NRT_EXEC_UNIT_UNRECOVERABLE) usually
        # clear on retry
        res = run_bass_kernel_spmd(nc, in_maps, list(range(NCORES)), trace=_trace)
    out = np.zeros((L, D), np.float64)
    for c in range(NCORES):
        out += res.results[c]["out_p"].astype(np.float64)
    if _trace:
        kernel._last_exec_time_ns = res.exec_time_ns
    return out.astype(np.float32).reshape(B, L, D)
